# revision 11
# baseline (speedup 1.0000x reference)
"""Trainium2 Bass kernel (fp8 TensorEngine path) for KnowledgeGraphEmbedding.

Per core (128 relations):
    diff[p,h] = [wi | -wj] . [L[p,h,:] | R[p,h,:]]   (600-dim contraction)
    dist[p]   = sum_h (diff/SCALE)^2
    out       = [dist*rel, dist*(1-rel), rel, 1-rel]

Strategy (memory-bound; fp8 quarters HBM traffic vs f32):
  - host: scale L,R by SCALE, quantize to fp8e4, transpose to
    contraction-major, pack for DoubleRow ([K,2,N] k-subtile pairs)
  - PE: per 512-column N-tile, 3 DoubleRow matmuls contract 600 dims
    (256+256+88) into psum [1, 512] f32
  - drains: DVE/ACT alternate copying psum -> staging rows [1, 12800]
  - reshape: 25-N-tile staging chunks DMA'd into diff [128, 300]
  - post: ACT Square(scale=1/SCALE) w/ accum -> dist; bins via DVE
"""

from contextlib import ExitStack

import numpy as np
import ml_dtypes

E4 = ml_dtypes.float8_e4m3

N_CORES = 8
P_TOTAL = 1024
H = 300
E = 300
ROW = 2 * E                 # merged [L-row | R-row] contraction dim
P_LOC = P_TOTAL // N_CORES  # 128
N = 512                     # psum bank columns per N-tile
NT = (P_LOC * H) // N       # 75 N-tiles per core
GRP = 3                     # N-tiles per DMA group
N_GRP = NT // GRP           # 25
CHUNKS = [(0, 128), (256, 128), (512, 44)]  # (row offset, k) DoubleRow pairs
SCALE = 32.0
STAGE_NT = 15               # N-tiles per staging chunk
STAGE_W = STAGE_NT * N      # 7680

TRACE = False
LAST_RESULT = None
USE_GPSIMD = False          # SWDGE ring for x2/reshape DMAs (crashes exec unit?)

_CACHE: dict = {}


def _stage_dmas(nc, diff, stage, k, eng):
    """Emit DMAs moving staging chunk k ([1, 12800] f32, global columns
    [k*12800, (k+1)*12800) in p-major (p,h) order) into diff [128, 300]."""
    g0 = k * STAGE_W
    pos = 0
    while pos < STAGE_W:
        g = g0 + pos
        p, h = divmod(g, H)
        if h != 0:
            run = min(H - h, STAGE_W - pos)
            eng.dma_start(diff[p : p + 1, h : h + run], stage[0:1, pos : pos + run])
            pos += run
            continue
        rows = (STAGE_W - pos) // H
        if rows == 0:
            run = STAGE_W - pos
            eng.dma_start(diff[p : p + 1, 0:run], stage[0:1, pos : pos + run])
            pos += run
        else:
            eng.dma_start(diff[p : p + rows, 0:H], stage[0:1, pos : pos + rows * H])
            pos += rows * H


def _build_nc():
    import concourse.bacc as bacc
    import concourse.mybir as mybir
    import concourse.tile as tile

    f32 = mybir.dt.float32
    fp8 = mybir.dt.float8e4

    nc = bacc.Bacc("TRN2", debug=False)

    x0 = nc.dram_tensor("x0", [128, NT * 1024], fp8, kind="ExternalInput").ap()
    x1 = nc.dram_tensor("x1", [128, NT * 1024], fp8, kind="ExternalInput").ap()
    x2 = nc.dram_tensor("x2", [44, NT * 1024], fp8, kind="ExternalInput").ap()
    wd = nc.dram_tensor("wd", [128, 96], fp8, kind="ExternalInput").ap()
    rm = nc.dram_tensor("rm", [P_LOC, 2], f32, kind="ExternalInput").ap()
    out = nc.dram_tensor("out", [P_LOC, 4], f32, kind="ExternalOutput").ap()

    with tile.TileContext(nc) as tc, ExitStack() as ctx:
        const_pool = ctx.enter_context(tc.tile_pool(name="const", bufs=1))
        data_pool = ctx.enter_context(tc.tile_pool(name="data", bufs=6))
        stage_pool = ctx.enter_context(tc.tile_pool(name="stage", bufs=2))
        psum_pool = ctx.enter_context(
            tc.tile_pool(name="ps", bufs=8, space=mybir.MemorySpace.PSUM)
        )

        wd_sb = const_pool.tile([128, 96], fp8)
        nc.sync.dma_start(wd_sb[:], wd[:])
        wd_v = wd_sb.rearrange("p (c s x) -> p c s x", c=3, s=2)  # x = 16
        rm_sb = const_pool.tile([P_LOC, 2], f32)
        nc.sync.dma_start(rm_sb[:], rm[:])

        diff = const_pool.tile([P_LOC, H], f32)

        stage = None
        for g in range(N_GRP):
            a0 = data_pool.tile([128, GRP * 1024], fp8)
            nc.sync.dma_start(a0[:], x0[:, g * GRP * 1024 : (g + 1) * GRP * 1024])
            a1 = data_pool.tile([128, GRP * 1024], fp8)
            nc.scalar.dma_start(a1[:], x1[:, g * GRP * 1024 : (g + 1) * GRP * 1024])
            a2 = data_pool.tile([44, GRP * 1024], fp8)
            eng2 = nc.gpsimd if USE_GPSIMD else (nc.scalar if g % 2 == 0 else nc.sync)
            eng2.dma_start(a2[:], x2[:, g * GRP * 1024 : (g + 1) * GRP * 1024])
            tiles = (a0, a1, a2)
            for i in range(GRP):
                m = g * GRP + i
                if m % STAGE_NT == 0:
                    stage = stage_pool.tile([1, STAGE_W], f32)
                ps = psum_pool.tile([1, N], f32)
                for c, (_, ksz) in enumerate(CHUNKS):
                    rhs = tiles[c][0:ksz, i * 1024 : (i + 1) * 1024].rearrange(
                        "p (s n) -> p s n", s=2
                    )
                    nc.tensor.matmul(
                        ps[:],
                        lhsT=wd_v[0:ksz, c, :, 0:1],
                        rhs=rhs,
                        start=(c == 0),
                        stop=(c == 2),
                        perf_mode=mybir.MatmulPerfMode.DoubleRow,
                    )
                dst = stage[0:1, (m % STAGE_NT) * N : (m % STAGE_NT + 1) * N]
                if m % 2 == 0:
                    nc.vector.tensor_copy(dst, ps[:])
                else:
                    nc.scalar.copy(dst, ps[:])
                if m % STAGE_NT == STAGE_NT - 1:
                    k = m // STAGE_NT
                    _stage_dmas(nc, diff, stage, k,
                                nc.sync if k % 2 == 0 else nc.scalar)

        dist = const_pool.tile([P_LOC, 1], f32)
        sq = const_pool.tile([P_LOC, H], f32)
        nc.scalar.activation(
            sq[:],
            diff[:],
            mybir.ActivationFunctionType.Square,
            scale=1.0 / SCALE,
            accum_out=dist[:],
        )

        out_sb = const_pool.tile([P_LOC, 4], f32)
        nc.vector.tensor_scalar_mul(out_sb[:, 0:2], rm_sb[:, 0:2], dist[:, 0:1])
        nc.vector.tensor_copy(out_sb[:, 2:4], rm_sb[:, 0:2])
        nc.sync.dma_start(out[:], out_sb[:])

    nc.compile()
    return nc


def _pack_chunk(V, off, ksz):
    """V: [NT*N, 600] fp8 (columns = contraction). Returns [ksz, NT*1024]
    with element [k, m*1024 + s*512 + n] = V[m*512+n, off + s*ksz + k]."""
    block = V[:, off : off + 2 * ksz]             # [38400, 2*ksz]
    arr = block.reshape(NT, N, 2, ksz)            # [m, n, s, k]
    return np.ascontiguousarray(arr.transpose(3, 0, 2, 1)).reshape(ksz, NT * 1024)


def kernel(tag_rep, Lp_w, Rp_w, relation, tag1_idx, tag2_idx):
    global LAST_RESULT
    from concourse.bass_utils import run_bass_kernel_spmd

    if "nc" not in _CACHE:
        _CACHE["nc"] = _build_nc()
    nc = _CACHE["nc"]

    tag_rep = np.asarray(tag_rep)
    Lp_w = np.asarray(Lp_w)
    Rp_w = np.asarray(Rp_w)
    rel = np.asarray(relation).astype(np.float32)

    wi = tag_rep[int(tag1_idx)].astype(np.float32)
    wj = tag_rep[int(tag2_idx)].astype(np.float32)
    u = np.concatenate([wi, -wj]).astype(E4)  # [600]
    wd_host = np.zeros((128, 96), dtype=E4)
    for c, (off, ksz) in enumerate(CHUNKS):
        wd_host[0:ksz, 32 * c] = u[off : off + ksz]
        wd_host[0:ksz, 32 * c + 16] = u[off + ksz : off + 2 * ksz]

    # Merged, scaled, quantized stream: [P, H, 600] fp8.
    Q = np.empty((P_TOTAL, H, ROW), dtype=E4)
    Q[:, :, :E] = Lp_w * SCALE
    Q[:, :, E:] = Rp_w * SCALE

    in_maps = []
    for ci in range(N_CORES):
        sl = slice(ci * P_LOC, (ci + 1) * P_LOC)
        V = Q[sl].reshape(P_LOC * H, ROW)  # rows = (p,h) p-major, cols = contraction
        rel_c = rel[sl]
        in_maps.append(
            {
                "x0": _pack_chunk(V, *CHUNKS[0]),
                "x1": _pack_chunk(V, *CHUNKS[1]),
                "x2": _pack_chunk(V, *CHUNKS[2]),
                "wd": wd_host,
                "rm": np.ascontiguousarray(np.stack([rel_c, 1.0 - rel_c], axis=1)),
            }
        )

    kw = {}
    if TRACE:
        kw = dict(trace=True, trace_cores=[0])
    res = run_bass_kernel_spmd(nc, in_maps, core_ids=list(range(N_CORES)), **kw)
    LAST_RESULT = res

    out_full = np.empty((4, P_TOTAL), dtype=np.float32)
    for c in range(N_CORES):
        out_full[:, c * P_LOC : (c + 1) * P_LOC] = res.results[c]["out"].T
    return out_full


# revision 12
# speedup vs baseline: 1.1226x; 1.1226x over previous
"""Trainium2 Bass kernel (fp8 TensorEngine path) for KnowledgeGraphEmbedding.

Per core (128 relations):
    diff[p,h] = [wi | -wj] . [L[p,h,:] | R[p,h,:]]   (600-dim contraction)
    dist[p]   = sum_h (diff/SCALE)^2
    out       = [dist*rel, dist*(1-rel), rel, 1-rel]

Strategy (memory-bound; fp8 quarters HBM traffic vs f32):
  - host: scale L,R by SCALE, quantize to fp8e4, transpose to
    contraction-major, pack for DoubleRow ([K,2,N] k-subtile pairs)
  - PE: per 512-column N-tile, 3 DoubleRow matmuls contract 600 dims
    (256+256+88) into psum [1, 512] f32
  - drains: DVE/ACT alternate copying psum -> staging rows [1, 12800]
  - reshape: 25-N-tile staging chunks DMA'd into diff [128, 300]
  - post: ACT Square(scale=1/SCALE) w/ accum -> dist; bins via DVE
"""

from contextlib import ExitStack

import numpy as np
import ml_dtypes

E4 = ml_dtypes.float8_e4m3

N_CORES = 8
P_TOTAL = 1024
H = 300
E = 300
ROW = 2 * E                 # merged [L-row | R-row] contraction dim
P_LOC = P_TOTAL // N_CORES  # 128
N = 512                     # psum bank columns per N-tile
NT = (P_LOC * H) // N       # 75 N-tiles per core
GRP = 5                     # N-tiles per DMA group
N_GRP = NT // GRP           # 15
CHUNKS = [(0, 128), (256, 128), (512, 44)]  # (row offset, k) DoubleRow pairs
SCALE = 32.0
STAGE_NT = 15               # N-tiles per staging chunk
STAGE_W = STAGE_NT * N      # 7680

TRACE = False
LAST_RESULT = None
USE_GPSIMD = False          # SWDGE ring for x2/reshape DMAs (crashes exec unit?)

_CACHE: dict = {}


def _stage_dmas(nc, diff, stage, k, eng):
    """Emit DMAs moving staging chunk k ([1, 12800] f32, global columns
    [k*12800, (k+1)*12800) in p-major (p,h) order) into diff [128, 300]."""
    g0 = k * STAGE_W
    pos = 0
    while pos < STAGE_W:
        g = g0 + pos
        p, h = divmod(g, H)
        if h != 0:
            run = min(H - h, STAGE_W - pos)
            eng.dma_start(diff[p : p + 1, h : h + run], stage[0:1, pos : pos + run])
            pos += run
            continue
        rows = (STAGE_W - pos) // H
        if rows == 0:
            run = STAGE_W - pos
            eng.dma_start(diff[p : p + 1, 0:run], stage[0:1, pos : pos + run])
            pos += run
        else:
            eng.dma_start(diff[p : p + rows, 0:H], stage[0:1, pos : pos + rows * H])
            pos += rows * H


def _build_nc():
    import concourse.bacc as bacc
    import concourse.mybir as mybir
    import concourse.tile as tile

    f32 = mybir.dt.float32
    fp8 = mybir.dt.float8e4

    nc = bacc.Bacc("TRN2", debug=False)

    x0 = nc.dram_tensor("x0", [128, NT * 1024], fp8, kind="ExternalInput").ap()
    x1 = nc.dram_tensor("x1", [128, NT * 1024], fp8, kind="ExternalInput").ap()
    x2 = nc.dram_tensor("x2", [44, NT * 1024], fp8, kind="ExternalInput").ap()
    wd = nc.dram_tensor("wd", [128, 96], fp8, kind="ExternalInput").ap()
    rm = nc.dram_tensor("rm", [P_LOC, 2], f32, kind="ExternalInput").ap()
    out = nc.dram_tensor("out", [P_LOC, 4], f32, kind="ExternalOutput").ap()

    with tile.TileContext(nc) as tc, ExitStack() as ctx:
        const_pool = ctx.enter_context(tc.tile_pool(name="const", bufs=1))
        data_pool = ctx.enter_context(tc.tile_pool(name="data", bufs=6))
        stage_pool = ctx.enter_context(tc.tile_pool(name="stage", bufs=2))
        psum_pool = ctx.enter_context(
            tc.tile_pool(name="ps", bufs=8, space=mybir.MemorySpace.PSUM)
        )

        wd_sb = const_pool.tile([128, 96], fp8)
        nc.sync.dma_start(wd_sb[:], wd[:])
        wd_v = wd_sb.rearrange("p (c s x) -> p c s x", c=3, s=2)  # x = 16
        rm_sb = const_pool.tile([P_LOC, 2], f32)
        nc.sync.dma_start(rm_sb[:], rm[:])

        diff = const_pool.tile([P_LOC, H], f32)

        stage = None
        for g in range(N_GRP):
            a0 = data_pool.tile([128, GRP * 1024], fp8)
            nc.sync.dma_start(a0[:], x0[:, g * GRP * 1024 : (g + 1) * GRP * 1024])
            a1 = data_pool.tile([128, GRP * 1024], fp8)
            nc.scalar.dma_start(a1[:], x1[:, g * GRP * 1024 : (g + 1) * GRP * 1024])
            a2 = data_pool.tile([44, GRP * 1024], fp8)
            eng2 = nc.gpsimd if USE_GPSIMD else (nc.scalar if g % 2 == 0 else nc.sync)
            eng2.dma_start(a2[:], x2[:, g * GRP * 1024 : (g + 1) * GRP * 1024])
            tiles = (a0, a1, a2)
            for i in range(GRP):
                m = g * GRP + i
                if m % STAGE_NT == 0:
                    stage = stage_pool.tile([1, STAGE_W], f32)
                ps = psum_pool.tile([1, N], f32)
                for c, (_, ksz) in enumerate(CHUNKS):
                    rhs = tiles[c][0:ksz, i * 1024 : (i + 1) * 1024].rearrange(
                        "p (s n) -> p s n", s=2
                    )
                    nc.tensor.matmul(
                        ps[:],
                        lhsT=wd_v[0:ksz, c, :, 0:1],
                        rhs=rhs,
                        start=(c == 0),
                        stop=(c == 2),
                        perf_mode=mybir.MatmulPerfMode.DoubleRow,
                    )
                dst = stage[0:1, (m % STAGE_NT) * N : (m % STAGE_NT + 1) * N]
                if m % 2 == 0:
                    nc.vector.tensor_copy(dst, ps[:])
                else:
                    nc.scalar.copy(dst, ps[:])
                if m % STAGE_NT == STAGE_NT - 1:
                    k = m // STAGE_NT
                    _stage_dmas(nc, diff, stage, k,
                                nc.sync if k % 2 == 0 else nc.scalar)

        dist = const_pool.tile([P_LOC, 1], f32)
        sq = const_pool.tile([P_LOC, H], f32)
        nc.scalar.activation(
            sq[:],
            diff[:],
            mybir.ActivationFunctionType.Square,
            scale=1.0 / SCALE,
            accum_out=dist[:],
        )

        out_sb = const_pool.tile([P_LOC, 4], f32)
        nc.vector.tensor_scalar_mul(out_sb[:, 0:2], rm_sb[:, 0:2], dist[:, 0:1])
        nc.vector.tensor_copy(out_sb[:, 2:4], rm_sb[:, 0:2])
        nc.sync.dma_start(out[:], out_sb[:])

    nc.compile()
    return nc


def _pack_chunk(V, off, ksz):
    """V: [NT*N, 600] fp8 (columns = contraction). Returns [ksz, NT*1024]
    with element [k, m*1024 + s*512 + n] = V[m*512+n, off + s*ksz + k]."""
    block = V[:, off : off + 2 * ksz]             # [38400, 2*ksz]
    arr = block.reshape(NT, N, 2, ksz)            # [m, n, s, k]
    return np.ascontiguousarray(arr.transpose(3, 0, 2, 1)).reshape(ksz, NT * 1024)


def kernel(tag_rep, Lp_w, Rp_w, relation, tag1_idx, tag2_idx):
    global LAST_RESULT
    from concourse.bass_utils import run_bass_kernel_spmd

    if "nc" not in _CACHE:
        _CACHE["nc"] = _build_nc()
    nc = _CACHE["nc"]

    tag_rep = np.asarray(tag_rep)
    Lp_w = np.asarray(Lp_w)
    Rp_w = np.asarray(Rp_w)
    rel = np.asarray(relation).astype(np.float32)

    wi = tag_rep[int(tag1_idx)].astype(np.float32)
    wj = tag_rep[int(tag2_idx)].astype(np.float32)
    u = np.concatenate([wi, -wj]).astype(E4)  # [600]
    wd_host = np.zeros((128, 96), dtype=E4)
    for c, (off, ksz) in enumerate(CHUNKS):
        wd_host[0:ksz, 32 * c] = u[off : off + ksz]
        wd_host[0:ksz, 32 * c + 16] = u[off + ksz : off + 2 * ksz]

    # Merged, scaled, quantized stream: [P, H, 600] fp8.
    Q = np.empty((P_TOTAL, H, ROW), dtype=E4)
    Q[:, :, :E] = Lp_w * SCALE
    Q[:, :, E:] = Rp_w * SCALE

    in_maps = []
    for ci in range(N_CORES):
        sl = slice(ci * P_LOC, (ci + 1) * P_LOC)
        V = Q[sl].reshape(P_LOC * H, ROW)  # rows = (p,h) p-major, cols = contraction
        rel_c = rel[sl]
        in_maps.append(
            {
                "x0": _pack_chunk(V, *CHUNKS[0]),
                "x1": _pack_chunk(V, *CHUNKS[1]),
                "x2": _pack_chunk(V, *CHUNKS[2]),
                "wd": wd_host,
                "rm": np.ascontiguousarray(np.stack([rel_c, 1.0 - rel_c], axis=1)),
            }
        )

    kw = {}
    if TRACE:
        kw = dict(trace=True, trace_cores=[0])
    res = run_bass_kernel_spmd(nc, in_maps, core_ids=list(range(N_CORES)), **kw)
    LAST_RESULT = res

    out_full = np.empty((4, P_TOTAL), dtype=np.float32)
    for c in range(N_CORES):
        out_full[:, c * P_LOC : (c + 1) * P_LOC] = res.results[c]["out"].T
    return out_full
